# revision 1
# baseline (speedup 1.0000x reference)
"""Trainium2 Bass kernel for nn_DenseNet3D_89730456748628.

Reference structure (after dead-code elimination):
  - The reference builds seq (B=64, T=512, 192) and runs two BiGRUs with
    sequence axis = B (64 steps) and batch axis = T (512).  The decoder
    consumes only dec_h_all[:64] and y2 is discarded, so only batch
    columns t < 64 matter.  Those depend only on x[t, c, 8*s, h, w]
    (t, s < 64) -- 3.1 MB of the 100 MB input and an 8x compute cut.
  - Critical path: 64 gru1 steps, then 64 gru2 steps (f/b directions of
    each phase packed side-by-side in the 128 partitions), then a
    6-step decoder GRU on batch 64.

Device layout (single NeuronCore; the recurrence is latency-bound):
  - Gate tensors: [128 part = (dir, batch), free = gates].
  - h kept transposed ([hid, (dir,batch)]) as matmul stationary operand;
    each step ends with two PE transposes of h_new.
  - x@Wih parts + biases accumulate into the next step's PSUM banks
    during the current step's elementwise chain (PE prefill).  Each
    PSUM bank gets exactly one start=True per step (bank-wide
    has_written clear), everything after accumulates.
  - float32r matmuls (full PE rate), fp32 elementwise.
"""

import re
from contextlib import ExitStack

import ml_dtypes

import numpy as np

import concourse.bass as bass
import concourse.tile as tile
from concourse import mybir
from concourse.bass_utils import run_bass_kernel_spmd
from concourse.tile import ScopedClock
from bass_rust import VectorClock

F32 = mybir.dt.float32
BF16 = mybir.dt.bfloat16

H = 256          # GRU hidden
V = 56           # vocab / fc1 out
NB = 64          # batch (original T slots used)
NS = 64          # scan steps (original B)
G = 3 * H        # 768 gates

AF = mybir.ActivationFunctionType
OP = mybir.AluOpType


def _vc_ticks(vc):
    m = re.search(r"\[([0-9, ]*)\]", repr(vc))
    s = m.group(1).strip()
    return [int(x) for x in s.split(",")] if s else []


class SplitDrainTC(tile.TileContext):
    """TileContext adapted to the installed walrus, which rejects >2
    sync waits on any single instruction: excess waits are peeled onto
    same-engine NOPs at commit time, and the exit drain emits one wait
    per sync.nop."""

    MAX_WAITS = 1

    def _add_instruction(self, inst):
        si = getattr(inst, "sync_info", None)
        if si is not None and si.on_wait and len(si.on_wait) > self.MAX_WAITS:
            waits = list(si.on_wait)
            keep = waits[: self.MAX_WAITS]
            excess = waits[self.MAX_WAITS :]
            for i in range(0, len(excess), self.MAX_WAITS):
                nop = mybir.InstNoOp(
                    name=self.nc.get_next_instruction_name(),
                    engine=inst.engine,
                    bass_nofuse=True,
                    sync_info=mybir.SyncInfo(
                        on_wait=excess[i : i + self.MAX_WAITS], on_update=[]),
                )
                super()._add_instruction(nop)
            inst.sync_info = mybir.SyncInfo(on_wait=keep, on_update=si.on_update)
        super()._add_instruction(inst)

    def _drain_and_barrier(self, tick_clock, wait_clock):
        ticks = _vc_ticks(tick_clock.global_clock)
        for i, t in enumerate(ticks):
            if t > 0:
                single = VectorClock([t if j == i else 0 for j in range(len(ticks))])
                nop = self.nc.sync.nop(nofuse=True)
                wait_clock.add_sem_waits(nop.ins, ScopedClock({None: single}))
        self.nc.sync.drain()
        self.nc.all_engine_barrier()
        popped = self.nc._tile_sem_poison_stack.pop()
        assert popped is self._sem_poison
        self.nc.clear_and_free_semaphores(list(self.sems.allocated().values()))
        self.nc.all_engine_barrier()


# ---------------------------------------------------------------------------
# host-side input preparation
# ---------------------------------------------------------------------------

def prepare_inputs(inputs, nsteps=NS):
    p = {k: np.asarray(v, dtype=np.float32) for k, v in inputs.items()
         if k != "target_length"}
    x = p["x"]

    # seq'[s, t, (c,h,w)] = x[t, c, 8s, h, w];  seqT[(c,h,w), s*64+t]
    xs = x[0:NB, :, 0 : 8 * nsteps : 8, :, :]            # [t, c, s, h, w]
    seqT = np.transpose(xs, (1, 3, 4, 2, 0)).reshape(192, nsteps * NB)
    seqT = np.concatenate([seqT, np.ones((1, nsteps * NB), np.float32)], 0)

    d = {"seqT": np.ascontiguousarray(seqT)}

    def gru_parts(tag, wih, whh, bih, bhh, aug):
        if aug:
            rz = np.concatenate([wih[:512].T, (bih[:512] + bhh[:512])[None, :]], 0)
            nn_ = np.concatenate([wih[512:].T, bih[512:][None, :]], 0)
        else:
            rz = wih[:512].T
            nn_ = wih[512:].T
            d[f"brz{tag}"] = np.ascontiguousarray((bih[:512] + bhh[:512])[None, :])
            d[f"bgin{tag}"] = np.ascontiguousarray(bih[512:][None, :])
        d[f"wihrz{tag}"] = np.ascontiguousarray(rz)
        d[f"wihn{tag}"] = np.ascontiguousarray(nn_)
        d[f"whh{tag}"] = np.ascontiguousarray(whh.T)
        d[f"bhhn{tag}"] = np.ascontiguousarray(bhh[512:][None, :])

    gru_parts("1f", p["w_ih_1f"], p["w_hh_1f"], p["b_ih_1f"], p["b_hh_1f"], True)
    gru_parts("1b", p["w_ih_1b"], p["w_hh_1b"], p["b_ih_1b"], p["b_hh_1b"], True)
    gru_parts("2f", p["w_ih_2f"], p["w_hh_2f"], p["b_ih_2f"], p["b_hh_2f"], False)
    gru_parts("2b", p["w_ih_2b"], p["w_hh_2b"], p["b_ih_2b"], p["b_hh_2b"], False)
    gru_parts("d", p["w_ih_d"], p["w_hh_d"], p["b_ih_d"], p["b_hh_d"], False)

    d["wadjT"] = np.ascontiguousarray(p["w_adj"].T)        # [512, 256]
    d["badj"] = np.ascontiguousarray(p["b_adj"][None, :])  # [1, 256]
    d["wfc1T"] = np.ascontiguousarray(p["w_fc1"].T)        # [256, 56]
    d["bfc1"] = np.ascontiguousarray(p["b_fc1"][None, :])  # [1, 56]
    d["ones"] = np.ones((1, 128), np.float32)
    d["zeros"] = np.zeros((128, 128), np.float32)
    f32_keep = {"ident"}
    for k in list(d):
        if k not in f32_keep:
            d[k] = d[k].astype(ml_dtypes.bfloat16)
    d["ident"] = np.eye(128, dtype=np.float32)
    d["identr"] = np.eye(128, dtype=np.float32)
    return d


# ---------------------------------------------------------------------------
# device program
# ---------------------------------------------------------------------------

class _HTView:
    """lhsT provider for packed f/b steps (slices of y1 storage)."""

    def __init__(self, ftile, btile, sf, sb):
        self.ftile, self.btile, self.sf, self.sb = ftile, btile, sf, sb

    def dir_ap(self, d):
        if d == 0:
            return self.ftile[:, self.sf : self.sf + 64]
        return self.btile[:, self.sb : self.sb + 64]


class _HTPlain:
    def __init__(self, t):
        self.t = t

    def dir_ap(self, d):
        return self.t[:, 0:64] if d == 0 else self.t[:, 64:128]


def build_program(nsteps=NS, tl=6):
    nc = bass.Bass("TRN2", target_bir_lowering=False, debug=False)
    SN = nsteps * NB

    dp = {}

    def din(name, shape, dtype=BF16):
        dp[name] = nc.declare_dram_parameter(name, list(shape), dtype, isOutput=False)

    din("seqT", (193, SN))
    for tag, Ka in (("1f", 193), ("1b", 193), ("2f", 512), ("2b", 512), ("d", 56)):
        din(f"wihrz{tag}", (Ka, 512))
        din(f"wihn{tag}", (Ka, 256))
        din(f"whh{tag}", (256, G))
        din(f"bhhn{tag}", (1, 256))
    for tag in ("2f", "2b", "d"):
        din(f"brz{tag}", (1, 512))
        din(f"bgin{tag}", (1, 256))
    din("wadjT", (512, 256))
    din("badj", (1, 256))
    din("wfc1T", (256, V))
    din("bfc1", (1, V))
    din("ones", (1, 128))
    din("zeros", (128, 128))
    din("ident", (128, 128), F32)
    din("identr", (128, 128))

    out_dram = nc.declare_dram_parameter("out", [tl, NB, V], F32, isOutput=True)

    with SplitDrainTC(nc) as tc:
        es = ExitStack()
        cpool = es.enter_context(tc.tile_pool(name="consts", bufs=1))

        def load(name, shape, dtype=BF16, src=None):
            t = cpool.tile(list(shape), dtype, tag=name)
            nc.sync.dma_start(out=t[:], in_=src if src is not None else dp[name][:])
            return t

        seqT0 = load("seqT0", (128, SN), src=dp["seqT"][0:128, :])
        seqT1 = load("seqT1", (65, SN), src=dp["seqT"][128:193, :])
        W = {}
        for tag, Ka in (("1f", 193), ("1b", 193), ("2f", 512), ("2b", 512), ("d", 56)):
            ks = [(i, min(128, Ka - i * 128)) for i in range((Ka + 127) // 128)]
            W[f"wihrz{tag}"] = [
                load(f"wihrz{tag}_{i}", (kn, 512),
                     src=dp[f"wihrz{tag}"][i * 128 : i * 128 + kn, :]) for i, kn in ks]
            W[f"wihn{tag}"] = [
                load(f"wihn{tag}_{i}", (kn, 256),
                     src=dp[f"wihn{tag}"][i * 128 : i * 128 + kn, :]) for i, kn in ks]
            W[f"whh{tag}"] = [
                load(f"whh{tag}_{i}", (128, G),
                     src=dp[f"whh{tag}"][i * 128 : (i + 1) * 128, :]) for i in range(2)]
            W[f"bhhn{tag}"] = load(f"bhhn{tag}", (1, 256))
        for tag in ("2f", "2b", "d"):
            W[f"brz{tag}"] = load(f"brz{tag}", (1, 512))
            W[f"bgin{tag}"] = load(f"bgin{tag}", (1, 256))
        wadjT = [load(f"wadjT_{k}", (128, 256),
                      src=dp["wadjT"][k * 128 : (k + 1) * 128, :]) for k in range(4)]
        badj = load("badj", (1, 256))
        wfc1T = [load(f"wfc1T_{k}", (128, V),
                      src=dp["wfc1T"][k * 128 : (k + 1) * 128, :]) for k in range(2)]
        bfc1 = load("bfc1", (1, V))
        ones = load("ones", (1, 128))
        ident = load("ident", (128, 128), F32)
        identr = load("identr", (128, 128))

        y1fT = [cpool.tile([128, SN], BF16, tag=f"y1fT{k}", name=f"y1fT{k}") for k in range(2)]
        y1bT = [cpool.tile([128, SN], BF16, tag=f"y1bT{k}", name=f"y1bT{k}") for k in range(2)]

        hT_init = [load(f"hTi{k}", (128, 128), src=dp["zeros"][:])
                   for k in range(2)]
        hA_init = cpool.tile([128, H], F32, tag="hAi", name="hAi")
        nc.vector.memset(hA_init[:], 0.0)
        hA_dec = cpool.tile([64, H], F32, tag="hAdec", name="hAdec")

        ppr = es.enter_context(tc.tile_pool(name="ppr", bufs=2, space="PSUM"))
        ppnz = es.enter_context(tc.tile_pool(name="ppnz", bufs=2, space="PSUM"))
        ppg = es.enter_context(tc.tile_pool(name="ppg", bufs=2, space="PSUM"))
        ptr = es.enter_context(tc.tile_pool(name="ptr", bufs=1, space="PSUM"))
        wrk = es.enter_context(tc.tile_pool(name="wrk", bufs=2))
        h2pool = es.enter_context(tc.tile_pool(name="h2T", bufs=2))

        def alloc_psum():
            return dict(
                pr=ppr.tile([128, 256], F32, tag="pr", name="pr",
                            padded_shape=[128, 512]),
                pnz=ppnz.tile([128, 512], F32, tag="pnz", name="pnz"),
                pg=ppg.tile([128, 256], F32, tag="pg", name="pg",
                            padded_shape=[128, 512]),
            )

        def emit_xpart_gru1(ps, s):
            """x-part + biases for gru1 step s (both dirs, prefill)."""
            for d in (0, 1):
                tag = "1f" if d == 0 else "1b"
                sl = s if d == 0 else (nsteps - 1 - s)
                c0, c1 = (0, 64) if d == 0 else (64, 128)
                tp = (0, c0)
                lhs = [seqT0[:, sl * 64 : sl * 64 + 64],
                       seqT1[:, sl * 64 : sl * 64 + 64]]
                for ki, lt in enumerate(lhs):
                    st = ki == 0
                    nc.tensor.matmul(ps["pr"][c0:c1, :], lt,
                                     W[f"wihrz{tag}"][ki][:, 0:256],
                                     start=st, stop=False, tile_position=tp,
                                     skip_group_check=(c0 == 64))
                    nc.tensor.matmul(ps["pnz"][c0:c1, 256:512], lt,
                                     W[f"wihrz{tag}"][ki][:, 256:512],
                                     start=st, stop=False, tile_position=tp,
                                     skip_group_check=(c0 == 64))
                    nc.tensor.matmul(ps["pg"][c0:c1, :], lt, W[f"wihn{tag}"][ki][:],
                                     start=st, stop=(ki == 1), tile_position=tp,
                                     skip_group_check=(c0 == 64))
                # bhh_n into pnz n-half (bank already started)
                nc.tensor.matmul(ps["pnz"][c0:c1, 0:256], ones[0:1, c0:c1],
                                 W[f"bhhn{tag}"][:],
                                 start=False, stop=False, tile_position=tp,
                                 skip_group_check=(c0 == 64))

        def emit_xpart_gru2(ps, s):
            for d in (0, 1):
                tag = "2f" if d == 0 else "2b"
                sl = s if d == 0 else (nsteps - 1 - s)
                c0, c1 = (0, 64) if d == 0 else (64, 128)
                tp = (0, c0)
                lhs = [y1fT[0][:, sl * 64 : sl * 64 + 64],
                       y1fT[1][:, sl * 64 : sl * 64 + 64],
                       y1bT[0][:, sl * 64 : sl * 64 + 64],
                       y1bT[1][:, sl * 64 : sl * 64 + 64]]
                for ki, lt in enumerate(lhs):
                    st = ki == 0
                    nc.tensor.matmul(ps["pr"][c0:c1, :], lt,
                                     W[f"wihrz{tag}"][ki][:, 0:256],
                                     start=st, stop=False, tile_position=tp,
                                     skip_group_check=(c0 == 64))
                    nc.tensor.matmul(ps["pnz"][c0:c1, 256:512], lt,
                                     W[f"wihrz{tag}"][ki][:, 256:512],
                                     start=st, stop=False, tile_position=tp,
                                     skip_group_check=(c0 == 64))
                    nc.tensor.matmul(ps["pg"][c0:c1, :], lt, W[f"wihn{tag}"][ki][:],
                                     start=st, stop=False, tile_position=tp,
                                     skip_group_check=(c0 == 64))
                on = ones[0:1, c0:c1]
                nc.tensor.matmul(ps["pr"][c0:c1, :], on, W[f"brz{tag}"][0:1, 0:256],
                                 start=False, stop=False, tile_position=tp,
                                 skip_group_check=(c0 == 64))
                nc.tensor.matmul(ps["pnz"][c0:c1, 256:512], on,
                                 W[f"brz{tag}"][0:1, 256:512],
                                 start=False, stop=False, tile_position=tp,
                                 skip_group_check=(c0 == 64))
                nc.tensor.matmul(ps["pg"][c0:c1, :], on, W[f"bgin{tag}"][:],
                                 start=False, stop=True, tile_position=tp,
                                 skip_group_check=(c0 == 64))
                nc.tensor.matmul(ps["pnz"][c0:c1, 0:256], on, W[f"bhhn{tag}"][:],
                                 start=False, stop=False, tile_position=tp,
                                 skip_group_check=(c0 == 64))

        def emit_hpart(ps, hT, tag_f, tag_b):
            """recurrent matmuls, chunk order r, n, z (both dirs col-tiled).

            Each psum bank's accumulation group (opened by the x-part
            prefill with start=True) is closed here: pr by the r chunk,
            pnz by the z chunk (its last write).
            """
            chunks = ((slice(0, 256), "pr", slice(0, 256), True),
                      (slice(512, 768), "pnz", slice(0, 256), False),
                      (slice(256, 512), "pnz", slice(256, 512), True))
            for wcols, pname, pcols, last in chunks:
                for d in (0, 1):
                    tag = tag_f if d == 0 else tag_b
                    c0, c1 = (0, 64) if d == 0 else (64, 128)
                    for ki in range(2):
                        nc.tensor.matmul(
                            ps[pname][c0:c1, pcols], hT[ki].dir_ap(d),
                            W[f"whh{tag}"][ki][:, wcols],
                            start=False, stop=(last and ki == 1),
                            tile_position=(0, c0),
                            skip_group_check=(c0 == 64))

        def emit_chain(ps, hA_prev, np_=128):
            r = wrk.tile([np_, 256], F32, tag="r", name="r", bufs=1)
            z = wrk.tile([np_, 256], F32, tag="z", name="z", bufs=1)
            tmp = wrk.tile([np_, 256], F32, tag="tmp", name="tmp", bufs=1)
            npre = wrk.tile([np_, 256], F32, tag="npre", name="npre", bufs=1)
            n = wrk.tile([np_, 256], F32, tag="n", name="n", bufs=1)
            u = wrk.tile([np_, 256], F32, tag="u", name="u", bufs=1)
            e = wrk.tile([np_, 256], F32, tag="e", name="e", bufs=1)
            f = wrk.tile([np_, 256], F32, tag="f", name="f", bufs=1)
            hn = wrk.tile([np_, 256], F32, tag="hn", name="hn")
            nc.scalar.activation(r[:], ps["pr"][0:np_, :], AF.Sigmoid)
            nc.scalar.activation(z[:], ps["pnz"][0:np_, 256:512], AF.Sigmoid)
            nc.vector.tensor_tensor(tmp[:], r[:], ps["pnz"][0:np_, 0:256], OP.mult)
            nc.vector.tensor_tensor(npre[:], tmp[:], ps["pg"][0:np_, :], OP.add)
            nc.scalar.activation(n[:], npre[:], AF.Tanh)
            nc.gpsimd.tensor_scalar(u[:], z[:], -1.0, 1.0, OP.mult, OP.add)
            nc.gpsimd.tensor_tensor(e[:], z[:], hA_prev[0:np_, :], OP.mult)
            nc.vector.tensor_tensor(f[:], u[:], n[:], OP.mult)
            nc.vector.tensor_tensor(hn[:], f[:], e[:], OP.add)
            return hn

        def emit_transp(hn, np_=128):
            t0 = ptr.tile([128, np_], F32, tag="t0", name="t0",
                          padded_shape=[128, 512])
            t1 = ptr.tile([128, np_], F32, tag="t1", name="t1",
                          padded_shape=[128, 512])
            nc.tensor.transpose(t0[:, 0:np_], hn[0:np_, 0:128], ident[0:np_, 0:np_])
            nc.tensor.transpose(t1[:, 0:np_], hn[0:np_, 128:256], ident[0:np_, 0:np_])
            return t0, t1

        # =================== phase 1: gru1 f+b ===========================
        ps_cur = alloc_psum()
        emit_xpart_gru1(ps_cur, 0)
        hT = [_HTPlain(hT_init[0]), _HTPlain(hT_init[1])]
        hA = hA_init
        for s in range(nsteps):
            emit_hpart(ps_cur, hT, "1f", "1b")
            ps_nxt = alloc_psum()
            if s + 1 < nsteps:
                emit_xpart_gru1(ps_nxt, s + 1)
            hn = emit_chain(ps_cur, hA)
            t0, t1 = emit_transp(hn)
            sf = s * 64
            sb = (nsteps - 1 - s) * 64
            nc.vector.tensor_copy(y1fT[0][:, sf : sf + 64], t0[:, 0:64])
            nc.scalar.copy(y1fT[1][:, sf : sf + 64], t1[:, 0:64])
            nc.vector.tensor_copy(y1bT[0][:, sb : sb + 64], t0[:, 64:128])
            nc.scalar.copy(y1bT[1][:, sb : sb + 64], t1[:, 64:128])
            hT = [_HTView(y1fT[k], y1bT[k], sf, sb) for k in range(2)]
            hA = hn
            ps_cur = ps_nxt

        # gru2 step 0 x-part: emitted after the final phase-1 copies (its
        # y1b slot 0 input is written by the last phase-1 step)
        emit_xpart_gru2(ps_cur, 0)

        # =================== phase 2: gru2 f+b ===========================
        hT = [_HTPlain(hT_init[0]), _HTPlain(hT_init[1])]
        hA = hA_init
        h2T_last = None
        for s in range(nsteps):
            emit_hpart(ps_cur, hT, "2f", "2b")
            if s + 1 < nsteps:
                ps_nxt = alloc_psum()
                emit_xpart_gru2(ps_nxt, s + 1)
            else:
                ps_nxt = None
            hn = emit_chain(ps_cur, hA)
            t0, t1 = emit_transp(hn)
            c0 = h2pool.tile([128, 128], BF16, tag="h2c0", name="h2c0")
            c1 = h2pool.tile([128, 128], BF16, tag="h2c1", name="h2c1")
            nc.vector.tensor_copy(c0[:], t0[:])
            nc.scalar.copy(c1[:], t1[:])
            hT = [_HTPlain(c0), _HTPlain(c1)]
            hA = hn
            h2T_last = (c0, c1)
            if ps_nxt is not None:
                ps_cur = ps_nxt

        # =================== decoder =====================================
        hc0, hc1 = h2T_last
        combT = [hc0[:, 0:64], hc1[:, 0:64], hc0[:, 64:128], hc1[:, 64:128]]
        dec_hT = []
        for m in range(2):
            pd = ptr.tile([128, 64], F32, tag="t0", name="t0",
                          padded_shape=[128, 512])
            for k in range(4):
                nc.tensor.matmul(pd[:, :], wadjT[k][:, m * 128 : (m + 1) * 128],
                                 combT[k], start=(k == 0), stop=False)
            nc.tensor.matmul(pd[:, :], badj[0:1, m * 128 : (m + 1) * 128],
                             ones[0:1, 0:64], start=False, stop=True)
            dh = h2pool.tile([128, 64], BF16, tag=f"dhT{m}", name=f"dhT{m}")
            nc.vector.tensor_copy(dh[:], pd[:])
            dec_hT.append(dh)
        # dec_h in chain layout, computed directly (combined @ w_adj^T + b_adj)
        pa = ptr.tile([128, 512], F32, tag="t1", name="t1",
                      padded_shape=[128, 512])
        for k in range(4):
            nc.tensor.matmul(pa[0:64, 0:256], combT[k], wadjT[k][:],
                             start=(k == 0), stop=False)
        nc.tensor.matmul(pa[0:64, 0:256], ones[0:1, 0:64], badj[:],
                         start=False, stop=True)
        nc.vector.tensor_copy(hA_dec[:, :], pa[0:64, 0:256])

        hT_d = dec_hT
        hA = hA_dec
        inpT = None
        for t in range(tl):
            ps = alloc_psum()
            on = ones[0:1, 0:64]
            if inpT is not None:
                nc.tensor.matmul(ps["pr"][0:64, :], inpT[:, :],
                                 W["wihrzd"][0][:, 0:256], start=True, stop=False)
                nc.tensor.matmul(ps["pnz"][0:64, 256:512], inpT[:, :],
                                 W["wihrzd"][0][:, 256:512], start=True, stop=False)
                nc.tensor.matmul(ps["pg"][0:64, :], inpT[:, :], W["wihnd"][0][:],
                                 start=True, stop=False)
                st = False
            else:
                st = True
            nc.tensor.matmul(ps["pr"][0:64, :], on, W["brzd"][0:1, 0:256],
                             start=st, stop=False)
            nc.tensor.matmul(ps["pnz"][0:64, 256:512], on, W["brzd"][0:1, 256:512],
                             start=st, stop=False)
            nc.tensor.matmul(ps["pg"][0:64, :], on, W["bgind"][:],
                             start=st, stop=True)
            nc.tensor.matmul(ps["pnz"][0:64, 0:256], on, W["bhhnd"][:],
                             start=False, stop=False)
            for ki in range(2):
                ht = hT_d[ki][:, 0:64]
                nc.tensor.matmul(ps["pr"][0:64, :], ht, W["whhd"][ki][:, 0:256],
                                 start=False, stop=(ki == 1))
                nc.tensor.matmul(ps["pnz"][0:64, 0:256], ht,
                                 W["whhd"][ki][:, 512:768], start=False, stop=False)
                nc.tensor.matmul(ps["pnz"][0:64, 256:512], ht,
                                 W["whhd"][ki][:, 256:512], start=False,
                                 stop=(ki == 1))
            hn = emit_chain(ps, hA, np_=64)
            t0, t1 = emit_transp(hn, np_=64)
            nh0 = h2pool.tile([128, 64], BF16, tag="dhT0", name="dhT0")
            nh1 = h2pool.tile([128, 64], BF16, tag="dhT1", name="dhT1")
            nc.vector.tensor_copy(nh0[:], t0[:])
            nc.scalar.copy(nh1[:], t1[:])
            hT_d = [nh0, nh1]
            hA = hn
            pf = ptr.tile([128, 64], F32, tag="t0", name="t0",
                          padded_shape=[128, 512])
            for ki in range(2):
                nc.tensor.matmul(pf[0:64, 0:V], hT_d[ki][:, 0:64], wfc1T[ki][:],
                                 start=(ki == 0), stop=False)
            nc.tensor.matmul(pf[0:64, 0:V], on, bfc1[:], start=False, stop=True)
            ob = wrk.tile([64, V], F32, tag="ob", name="ob", bufs=1)
            nc.vector.tensor_copy(ob[:], pf[0:64, 0:V])
            nc.sync.dma_start(out=out_dram[t], in_=ob[:])
            if t + 1 < tl:
                pi = ptr.tile([128, 64], F32, tag="t1", name="t1",
                          padded_shape=[128, 512])
                nc.tensor.transpose(pi[0:V, 0:64], ob[:, :], ident[0:64, 0:64])
                it = h2pool.tile([V, 64], BF16, tag="inpT", name="inpT")
                nc.vector.tensor_copy(it[:], pi[0:V, 0:64])
                inpT = it

        es.close()

    return nc


_PROG_CACHE = {}


def _get_program(nsteps, tl):
    key = (nsteps, tl)
    if key not in _PROG_CACHE:
        _PROG_CACHE[key] = build_program(nsteps, tl)
    return _PROG_CACHE[key]


def run_device(inputs, nsteps=NS, trace=False):
    tl = int(np.asarray(inputs["target_length"]))
    nc = _get_program(nsteps, tl)
    d = prepare_inputs(inputs, nsteps)
    res = run_bass_kernel_spmd(nc, [d], [0], trace=trace)
    out = res.results[0]["out"]          # [tl, 64, 56]
    full = np.ascontiguousarray(np.transpose(out, (1, 0, 2)).astype(np.float32))
    return full, res


def kernel(**inputs):
    return run_device(inputs)[0]



# revision 13
# speedup vs baseline: 1.3459x; 1.3459x over previous
"""Trainium2 Bass kernel for nn_DenseNet3D_89730456748628.

Reference structure (after dead-code elimination):
  - The reference builds seq (B=64, T=512, 192) and runs two BiGRUs with
    sequence axis = B (64 steps) and batch axis = T (512).  The decoder
    consumes only dec_h_all[:64] and y2 is discarded, so only batch
    columns t < 64 matter.  Those depend only on x[t, c, 8*s, h, w]
    (t, s < 64) -- 3.1 MB of the 100 MB input and an 8x compute cut.
  - Critical path: 64 gru1 steps, then 64 gru2 steps, then a 6-step
    decoder GRU on batch 64.

Layout (single NeuronCore; fully transposed: gates/hidden on
partitions, batch on the free axis):
  - Input-part GEMMs batched per layer/direction as weight-stationary
    GEMMs (lhsT = W chunk, M=128 gates; rhs = data, N=512 col tiles);
    per-gate biases folded into the PSUM->SBUF eviction via
    per-partition bias columns.  Eviction rotates ACT/DVE/Pool.
  - z-gate weights/biases negated host-side so sigmoid yields (1-z)
    directly; h' = h + (1-z)*(n - h).
  - Recurrent matmul split: Whh@h(s+1) = Whh@h(s) + Whh@g(s); the
    h-part runs one step early (off the critical cycle), only the 12
    g-part matmuls (rhs = last update g) sit on the per-step cycle.
  - Per-step PSUM bank (128,512) per direction:
    cols [0:256] = r|omz logits (x-inject + h/g mms),
    cols [256:384] = hn part (+b_hh_n ones-inject);
    the decoder also uses [384:512] for its x n-part.
  - No PE transposes anywhere; h' tiles are directly the next step's
    matmul rhs; gru1's h' writes ARE the y1T storage read by gru2's
    input GEMM.  bf16 matmuls + bf16 elementwise, fp32 PSUM.
"""

import re
from contextlib import ExitStack

import ml_dtypes

import numpy as np

import concourse.bass as bass
import concourse.tile as tile
from concourse import mybir
from concourse.bass_utils import run_bass_kernel_spmd
from concourse.tile import ScopedClock
from bass_rust import VectorClock

F32 = mybir.dt.float32
BF16 = mybir.dt.bfloat16

H = 256          # GRU hidden
V = 56           # vocab / fc1 out
NB = 64          # batch (original T slots used)
NS = 64          # scan steps (original B)
G = 3 * H        # 768 gates
SN = NS * NB     # 4096

AF = mybir.ActivationFunctionType
OP = mybir.AluOpType


def _vc_ticks(vc):
    m = re.search(r"\[([0-9, ]*)\]", repr(vc))
    s = m.group(1).strip()
    return [int(x) for x in s.split(",")] if s else []


class SplitDrainTC(tile.TileContext):
    """TileContext adapted to the installed walrus, which rejects >2
    sync waits on any single instruction: excess waits are peeled onto
    same-engine NOPs at commit time, and the exit drain emits one wait
    per sync.nop."""

    MAX_WAITS = 1

    def _add_instruction(self, inst):
        si = getattr(inst, "sync_info", None)
        if si is not None and si.on_wait and len(si.on_wait) > self.MAX_WAITS:
            waits = list(si.on_wait)
            keep = waits[: self.MAX_WAITS]
            excess = waits[self.MAX_WAITS :]
            for i in range(0, len(excess), self.MAX_WAITS):
                nop = mybir.InstNoOp(
                    name=self.nc.get_next_instruction_name(),
                    engine=inst.engine,
                    bass_nofuse=True,
                    sync_info=mybir.SyncInfo(
                        on_wait=excess[i : i + self.MAX_WAITS], on_update=[]),
                )
                super()._add_instruction(nop)
            inst.sync_info = mybir.SyncInfo(on_wait=keep, on_update=si.on_update)
        super()._add_instruction(inst)

    def _drain_and_barrier(self, tick_clock, wait_clock):
        ticks = _vc_ticks(tick_clock.global_clock)
        for i, t in enumerate(ticks):
            if t > 0:
                single = VectorClock([t if j == i else 0 for j in range(len(ticks))])
                nop = self.nc.sync.nop(nofuse=True)
                wait_clock.add_sem_waits(nop.ins, ScopedClock({None: single}))
        self.nc.sync.drain()
        self.nc.all_engine_barrier()
        popped = self.nc._tile_sem_poison_stack.pop()
        assert popped is self._sem_poison
        self.nc.clear_and_free_semaphores(list(self.sems.allocated().values()))
        self.nc.all_engine_barrier()


# ---------------------------------------------------------------------------
# host-side input preparation
# ---------------------------------------------------------------------------

def prepare_inputs(inputs, nsteps=NS):
    p = {k: np.asarray(v, dtype=np.float32) for k, v in inputs.items()
         if k != "target_length"}
    x = p["x"]

    # seq'[s, t, (c,h,w)] = x[t, c, 8s, h, w];  seqT[(c,h,w), s*64+t]
    xs = x[0:NB, :, 0 : 8 * nsteps : 8, :, :]            # [t, c, s, h, w]
    seqT = np.transpose(xs, (1, 3, 4, 2, 0)).reshape(192, nsteps * NB)

    d = {"seqT0": seqT[0:128].copy(), "seqT1": seqT[128:192].copy()}

    def gru_parts(tag, wih, whh, bih, bhh):
        wihT = wih.T.copy()
        whhT = whh.T.copy()
        # z-gate negation: sigmoid(-z_logit) = 1 - z
        wihT[:, 256:512] *= -1.0
        whhT[:, 256:512] *= -1.0
        evb = np.concatenate([
            bih[0:256] + bhh[0:256],
            -(bih[256:512] + bhh[256:512]),
            bih[512:768],
        ])
        d[f"wihT{tag}"] = wihT
        d[f"whhT{tag}"] = whhT
        d[f"evb{tag}"] = evb.reshape(6, 128).T.copy()      # [128, 6]
        d[f"bhhn{tag}"] = bhh[512:768].reshape(1, 256).copy()

    gru_parts("1f", p["w_ih_1f"], p["w_hh_1f"], p["b_ih_1f"], p["b_hh_1f"])
    gru_parts("1b", p["w_ih_1b"], p["w_hh_1b"], p["b_ih_1b"], p["b_hh_1b"])
    gru_parts("2f", p["w_ih_2f"], p["w_hh_2f"], p["b_ih_2f"], p["b_hh_2f"])
    gru_parts("2b", p["w_ih_2b"], p["w_hh_2b"], p["b_ih_2b"], p["b_hh_2b"])

    # decoder
    wihdT = p["w_ih_d"].T.copy()
    whhdT = p["w_hh_d"].T.copy()
    wihdT[:, 256:512] *= -1.0
    whhdT[:, 256:512] *= -1.0
    brzd = (p["b_ih_d"][0:512] + p["b_hh_d"][0:512]).copy()
    brzd[256:512] *= -1.0
    d["wihdT"] = wihdT
    d["whhdT"] = whhdT
    d["brzd"] = brzd.reshape(1, 512)
    d["bihdn"] = p["b_ih_d"][512:768].reshape(1, 256).copy()
    d["bhhdn"] = p["b_hh_d"][512:768].reshape(1, 256).copy()

    d["wadjT"] = p["w_adj"].T.copy()                       # [512, 256]
    d["badjc"] = p["b_adj"].reshape(2, 128).T.copy()       # [128, 2]
    d["wfc1T"] = p["w_fc1"].T.copy()                       # [256, 56]
    d["bfc1c"] = p["b_fc1"].reshape(56, 1).copy()          # [56, 1]
    d["ident"] = np.eye(128, dtype=np.float32)
    d["ones"] = np.ones((1, NB), np.float32)

    f32_keep = {"evb1f", "evb1b", "evb2f", "evb2b", "badjc", "bfc1c"}
    for k in list(d):
        if k not in f32_keep:
            d[k] = np.ascontiguousarray(d[k]).astype(ml_dtypes.bfloat16)
        else:
            d[k] = np.ascontiguousarray(d[k])
    return d


# ---------------------------------------------------------------------------
# device program
# ---------------------------------------------------------------------------

def build_program(nsteps=NS, tl=6):
    nc = bass.Bass("TRN2", target_bir_lowering=False, debug=False)
    sn = nsteps * NB

    dp = {}

    def din(name, shape, dtype=BF16):
        dp[name] = nc.declare_dram_parameter(name, list(shape), dtype, isOutput=False)

    din("seqT0", (128, sn))
    din("seqT1", (64, sn))
    for tag in ("1f", "1b"):
        din(f"wihT{tag}", (192, G))
    for tag in ("2f", "2b"):
        din(f"wihT{tag}", (512, G))
    for tag in ("1f", "1b", "2f", "2b"):
        din(f"whhT{tag}", (H, G))
        din(f"evb{tag}", (128, 6), F32)
        din(f"bhhn{tag}", (1, 256))
    din("wihdT", (V, G))
    din("whhdT", (H, G))
    din("brzd", (1, 512))
    din("bihdn", (1, 256))
    din("bhhdn", (1, 256))
    din("wadjT", (512, 256))
    din("badjc", (128, 2), F32)
    din("wfc1T", (256, V))
    din("bfc1c", (V, 1), F32)
    din("ident", (128, 128))
    din("ones", (1, NB))

    out_dram = nc.declare_dram_parameter("out", [tl, V, NB], F32, isOutput=True)

    with SplitDrainTC(nc) as tc:
        es = ExitStack()
        cpool = es.enter_context(tc.tile_pool(name="consts", bufs=1))

        def load(name, shape, dtype=BF16, src=None):
            t = cpool.tile(list(shape), dtype, tag=name, name=name)
            nc.sync.dma_start(out=t[:], in_=src if src is not None else dp[name][:])
            return t

        seqT = [load("seqT0", (128, sn)), load("seqT1", (64, sn))]
        WIH1 = {}
        for tag in ("1f", "1b"):
            WIH1[tag] = [
                load(f"wihT{tag}_0", (128, G), src=dp[f"wihT{tag}"][0:128, :]),
                load(f"wihT{tag}_1", (64, G), src=dp[f"wihT{tag}"][128:192, :]),
            ]
        WIH2 = {}
        for tag in ("2f", "2b"):
            WIH2[tag] = [
                load(f"wihT{tag}_{k}", (128, G),
                     src=dp[f"wihT{tag}"][k * 128:(k + 1) * 128, :])
                for k in range(4)
            ]
        WHH, EVB, BHHN = {}, {}, {}
        for tag in ("1f", "1b", "2f", "2b"):
            WHH[tag] = [
                load(f"whhT{tag}_{k}", (128, G),
                     src=dp[f"whhT{tag}"][k * 128:(k + 1) * 128, :])
                for k in range(2)
            ]
            EVB[tag] = load(f"evb{tag}", (128, 6), F32)
            BHHN[tag] = load(f"bhhn{tag}", (1, 256))
        WIHD = load("wihdT", (V, G))
        WHHD = [load(f"whhdT_{k}", (128, G),
                     src=dp["whhdT"][k * 128:(k + 1) * 128, :]) for k in range(2)]
        BRZD = load("brzd", (1, 512))
        BIHDN = load("bihdn", (1, 256))
        BHHDN = load("bhhdn", (1, 256))
        WADJ = [load(f"wadjT_{k}", (128, 256),
                     src=dp["wadjT"][k * 128:(k + 1) * 128, :]) for k in range(4)]
        BADJC = load("badjc", (128, 2), F32)
        WFC1 = [load(f"wfc1T_{k}", (128, V),
                     src=dp["wfc1T"][k * 128:(k + 1) * 128, :]) for k in range(2)]
        BFC1C = load("bfc1c", (V, 1), F32)
        IDENT = load("ident", (128, 128))
        ONES = load("ones", (1, NB))

        # big SBUF storage.  xTf/xTb are reused by both layers (the gru2
        # input GEMM overwrites them only after gru1's recurrence has
        # consumed them -- enforced by tile dependency tracking).
        xTf = cpool.tile([128, 6, sn], BF16, tag="xTf", name="xTf")
        xTb = cpool.tile([128, 6, sn], BF16, tag="xTb", name="xTb")
        y1T = cpool.tile([128, 4, sn], BF16, tag="y1T", name="y1T")
        zeros3 = cpool.tile([128, 2, NB], BF16, tag="zeros3", name="zeros3")
        nc.vector.memset(zeros3[:], 0.0)

        pstep = es.enter_context(tc.tile_pool(name="pstep", bufs=2, space="PSUM"))
        pgem = es.enter_context(tc.tile_pool(name="pgem", bufs=2, space="PSUM"))
        wrk = es.enter_context(tc.tile_pool(name="wrk", bufs=3))

        # ----------------------------------------------------------------
        # batched input-part GEMM:
        #   xdst[:, m, cb*512:...] = (sum_ki lhs[ki].T @ rhs(ki, cb))[Mtile m] + evb[:, m]
        # ----------------------------------------------------------------
        def xgemm(xdst, lhs_chunks, rhs_fn, evb, ecnt0=0):
            nblk = sn // 512
            ecnt = ecnt0
            for m in range(6):
                for cb in range(nblk):
                    pg = pgem.tile([128, 512], F32, tag="pg", name="pg")
                    nk = len(lhs_chunks)
                    for ki in range(nk):
                        nc.tensor.matmul(
                            pg[:], lhs_chunks[ki][:, m * 128:(m + 1) * 128],
                            rhs_fn(ki, cb),
                            start=(ki == 0), stop=(ki == nk - 1))
                    dst = xdst[:, m, cb * 512:(cb + 1) * 512]
                    e = ecnt % 2
                    ecnt += 1
                    if e == 0:
                        nc.scalar.activation(dst, pg[:], AF.Identity,
                                             bias=evb[:, m:m + 1])
                    else:
                        nc.vector.tensor_scalar(dst, pg[:], evb[:, m:m + 1],
                                                None, OP.add)
            return ecnt

        # ----------------------------------------------------------------
        # one BiGRU layer, transposed layout, g-split recurrence
        # ----------------------------------------------------------------
        def gru_layer(tag_f, tag_b, xT, y_store):
            """xT[d] = input-part tensor (128, 6, sn) for dir d.
            y_store: None -> rotating h tiles (gru2); else the y1T tile
            (gru1: h' writes double as the y1 storage).
            Returns per-dir objects with .full() / .rhs(ki) for the
            final hidden."""
            tags = (tag_f, tag_b)
            col = lambda d, s: s if d == 0 else nsteps - 1 - s

            class HRef:
                def __init__(self, tile_, d, c):
                    self.t, self.d, self.c = tile_, d, c

                def full(self):
                    if self.c is None:
                        return self.t[:]
                    return self.t[:, 2 * self.d:2 * self.d + 2,
                                  self.c * NB:(self.c + 1) * NB]

                def rhs(self, ki):
                    if self.c is None:
                        return self.t[:, ki, :]
                    return self.t[:, 2 * self.d + ki,
                                  self.c * NB:(self.c + 1) * NB]

            def make_hdst(d, s):
                if y_store is None:
                    t = wrk.tile([128, 2, NB], BF16, tag=f"h2_{d}",
                                 name=f"h2_{d}")
                    return HRef(t, d, None)
                return HRef(y_store, d, col(d, s))

            hmap = {}

            def inject(bank, d, s, start):
                c = col(d, s)
                nc.tensor.matmul(bank[:, 0:256], IDENT[:],
                                 xT[d][:, 0:4, c * NB:(c + 1) * NB],
                                 start=start, stop=False,
                                 skip_group_check=not start)

            def bhhn_mms(bank, d, stop):
                bt = BHHN[tags[d]]
                for ch in range(2):
                    nc.tensor.matmul(
                        bank[:, 256 + ch * NB:256 + (ch + 1) * NB],
                        bt[:, ch * 128:(ch + 1) * 128], ONES[:],
                        start=False, stop=(stop and ch == 1),
                        skip_group_check=True)

            def wh_mms(bank, d, rhs_fn, stop):
                w = WHH[tags[d]]
                for m in range(6):
                    c0 = m * NB if m < 4 else 256 + (m - 4) * NB
                    for ki in range(2):
                        nc.tensor.matmul(
                            bank[:, c0:c0 + NB],
                            w[ki][:, m * 128:(m + 1) * 128],
                            rhs_fn(ki),
                            start=False,
                            stop=(stop and m == 5 and ki == 1),
                            skip_group_check=True)

            banks = {}
            for d in (0, 1):
                b0 = pstep.tile([128, 512], F32, tag=f"A{d}", name=f"bank{d}")
                inject(b0, d, 0, start=True)
                bhhn_mms(b0, d, stop=True)
                banks[d] = b0
            gt = {0: None, 1: None}

            for s in range(nsteps):
                # close current banks with g-part matmuls
                if s >= 1:
                    for d in (0, 1):
                        gtile = gt[d]
                        wh_mms(banks[d], d, lambda ki, _t=gtile: _t[:, ki, :],
                               stop=True)
                # open next banks; h-part runs one step ahead
                nbanks = {}
                if s + 1 < nsteps:
                    for d in (0, 1):
                        nb = pstep.tile([128, 512], F32, tag=f"A{d}",
                                        name=f"bank{d}")
                        inject(nb, d, s + 1, start=True)
                        bhhn_mms(nb, d, stop=False)
                        if s >= 1:
                            wh_mms(nb, d, hmap[d].rhs, stop=False)
                        nbanks[d] = nb
                # elementwise chain
                rzs, tmp, npre, nt, dmn = {}, {}, {}, {}, {}
                for d in (0, 1):
                    rzs[d] = wrk.tile([128, 4, NB], BF16, tag=f"rzs{d}",
                                      name=f"rzs{d}")
                    nc.scalar.activation(rzs[d][:], banks[d][:, 0:256],
                                         AF.Sigmoid)
                for d in (0, 1):
                    tmp[d] = wrk.tile([128, 2, NB], BF16, tag=f"tmp{d}",
                                      name=f"tmp{d}")
                    nc.vector.tensor_tensor(tmp[d][:], rzs[d][:, 0:2, :],
                                            banks[d][:, 256:384], OP.mult)
                for d in (0, 1):
                    c = col(d, s)
                    npre[d] = wrk.tile([128, 2, NB], BF16, tag=f"npre{d}",
                                       name=f"npre{d}")
                    nc.vector.tensor_tensor(npre[d][:], tmp[d][:],
                                            xT[d][:, 4:6, c * NB:(c + 1) * NB],
                                            OP.add)
                for d in (0, 1):
                    nt[d] = wrk.tile([128, 2, NB], BF16, tag=f"nt{d}",
                                     name=f"nt{d}")
                    nc.scalar.activation(nt[d][:], npre[d][:], AF.Tanh)
                for d in (0, 1):
                    hc = zeros3[:] if s == 0 else hmap[d].full()
                    dmn[d] = wrk.tile([128, 2, NB], BF16, tag=f"dmn{d}",
                                      name=f"dmn{d}")
                    nc.vector.tensor_tensor(dmn[d][:], nt[d][:], hc, OP.subtract)
                for d in (0, 1):
                    g = wrk.tile([128, 2, NB], BF16, tag=f"g{d}", name=f"g{d}")
                    nc.vector.tensor_tensor(g[:], rzs[d][:, 2:4, :],
                                            dmn[d][:], OP.mult)
                    gt[d] = g
                for d in (0, 1):
                    hc = zeros3[:] if s == 0 else hmap[d].full()
                    hdst = make_hdst(d, s)
                    nc.gpsimd.tensor_tensor(hdst.full(), hc, gt[d][:], OP.add)
                    hmap[d] = hdst
                banks = nbanks
            return hmap

        # =================== phase 1: gru1 ==============================
        def seq_rhs(ki, cb):
            return seqT[ki][:, cb * 512:(cb + 1) * 512]

        ec = xgemm(xTf, WIH1["1f"], seq_rhs, EVB["1f"])
        ec = xgemm(xTb, WIH1["1b"], seq_rhs, EVB["1b"], ec)

        gru_layer("1f", "1b", (xTf, xTb), y1T)

        # =================== phase 2: gru2 ==============================
        def y1_rhs(ki, cb):
            return y1T[:, ki, cb * 512:(cb + 1) * 512]

        ec = xgemm(xTf, WIH2["2f"], y1_rhs, EVB["2f"], ec)
        ec = xgemm(xTb, WIH2["2b"], y1_rhs, EVB["2b"], ec)

        h2 = gru_layer("2f", "2b", (xTf, xTb), None)

        # =================== decoder ====================================
        # dec_h = [h2f, h2b] @ w_adj.T + b_adj   (transposed: (2x128, 64))
        pd = pgem.tile([128, 512], F32, tag="pg", name="pd")
        first = True
        for m in range(2):
            for ki in range(4):
                rhs = h2[0].rhs(ki) if ki < 2 else h2[1].rhs(ki - 2)
                nc.tensor.matmul(pd[:, m * NB:(m + 1) * NB],
                                 WADJ[ki][:, m * 128:(m + 1) * 128], rhs,
                                 start=first, stop=(m == 1 and ki == 3),
                                 skip_group_check=not first)
                first = False
        hdec = wrk.tile([128, 2, NB], BF16, tag="hdec", name="hdec")
        for m in range(2):
            nc.scalar.activation(hdec[:, m, :], pd[:, m * NB:(m + 1) * NB],
                                 AF.Identity, bias=BADJC[:, m:m + 1])
        hdec_t = hdec

        inT = None
        for t in range(tl):
            bank = pstep.tile([128, 512], F32, tag="A0", name="bankd")
            # bias injects (opener: first brzd mm)
            for m in range(4):
                nc.tensor.matmul(bank[:, m * NB:(m + 1) * NB],
                                 BRZD[:, m * 128:(m + 1) * 128], ONES[:],
                                 start=(m == 0), stop=False,
                                 skip_group_check=(m != 0))
            for ch in range(2):
                nc.tensor.matmul(bank[:, 384 + ch * NB:384 + (ch + 1) * NB],
                                 BIHDN[:, ch * 128:(ch + 1) * 128], ONES[:],
                                 start=False, stop=False, skip_group_check=True)
                nc.tensor.matmul(bank[:, 256 + ch * NB:256 + (ch + 1) * NB],
                                 BHHDN[:, ch * 128:(ch + 1) * 128], ONES[:],
                                 start=False, stop=False, skip_group_check=True)
            # x-part (input is zero at t=0)
            if inT is not None:
                for m in range(6):
                    c0 = m * NB if m < 4 else 384 + (m - 4) * NB
                    nc.tensor.matmul(bank[:, c0:c0 + NB],
                                     WIHD[:, m * 128:(m + 1) * 128], inT[:],
                                     start=False, stop=False,
                                     skip_group_check=True)
            # h-part
            for m in range(6):
                c0 = m * NB if m < 4 else 256 + (m - 4) * NB
                for ki in range(2):
                    nc.tensor.matmul(bank[:, c0:c0 + NB],
                                     WHHD[ki][:, m * 128:(m + 1) * 128],
                                     hdec_t[:, ki, :],
                                     start=False,
                                     stop=(m == 5 and ki == 1),
                                     skip_group_check=True)
            rzs = wrk.tile([128, 4, NB], BF16, tag="rzsd", name="rzsd")
            nc.scalar.activation(rzs[:], bank[:, 0:256], AF.Sigmoid)
            tmp = wrk.tile([128, 2, NB], BF16, tag="tmpd", name="tmpd")
            nc.vector.tensor_tensor(tmp[:], rzs[:, 0:2, :], bank[:, 256:384],
                                    OP.mult)
            npre = wrk.tile([128, 2, NB], BF16, tag="npred", name="npred")
            nc.vector.tensor_tensor(npre[:], tmp[:], bank[:, 384:512], OP.add)
            nt = wrk.tile([128, 2, NB], BF16, tag="ntd", name="ntd")
            nc.scalar.activation(nt[:], npre[:], AF.Tanh)
            dmn = wrk.tile([128, 2, NB], BF16, tag="dmnd", name="dmnd")
            nc.vector.tensor_tensor(dmn[:], nt[:], hdec_t[:], OP.subtract)
            g = wrk.tile([128, 2, NB], BF16, tag="gd", name="gd")
            nc.vector.tensor_tensor(g[:], rzs[:, 2:4, :], dmn[:], OP.mult)
            hnew = wrk.tile([128, 2, NB], BF16, tag="hnd", name="hnd")
            nc.gpsimd.tensor_tensor(hnew[:], hdec_t[:], g[:], OP.add)
            hdec_t = hnew

            # fc1: outT = w_fc1 @ h' + b   -> (56, 64)
            pf = pgem.tile([128, 512], F32, tag="pg", name="pf")
            for ki in range(2):
                nc.tensor.matmul(pf[0:V, 0:NB], WFC1[ki][:, 0:V],
                                 hnew[:, ki, :],
                                 start=(ki == 0), stop=(ki == 1))
            outf = wrk.tile([V, NB], F32, tag="outf", name="outf")
            nc.scalar.activation(outf[:], pf[0:V, 0:NB], AF.Identity,
                                 bias=BFC1C[:, 0:1])
            nc.sync.dma_start(out=out_dram[t], in_=outf[:])
            if t + 1 < tl:
                it = wrk.tile([V, NB], BF16, tag="inT", name="inT")
                nc.vector.tensor_copy(it[:], outf[:])
                inT = it

        es.close()

    return nc


_PROG_CACHE = {}


def _get_program(nsteps, tl):
    key = (nsteps, tl)
    if key not in _PROG_CACHE:
        _PROG_CACHE[key] = build_program(nsteps, tl)
    return _PROG_CACHE[key]


def run_device(inputs, nsteps=NS, trace=False):
    tl = int(np.asarray(inputs["target_length"]))
    nc = _get_program(nsteps, tl)
    d = prepare_inputs(inputs, nsteps)
    res = run_bass_kernel_spmd(nc, [d], [0], trace=trace)
    out = res.results[0]["out"]          # [tl, V, NB]
    full = np.ascontiguousarray(
        np.transpose(out, (2, 0, 1)).astype(np.float32))
    return full, res


def kernel(**inputs):
    return run_device(inputs)[0]
